# revision 9
# baseline (speedup 1.0000x reference)
"""VQ codebook (DKVB) kernel for Trainium2, sharded over 8 NeuronCores.

Problem: embeddings [8, 2048, 4, 4] -> tokens x [128, 256, 8]; per codebook c
(256 of them), find nearest code among 4096 (euclidean), gather values row.

Strategy: shard the 256 codebooks across 8 cores (32 per core). Each core:
  - x_aug^T  [32, 9, 128]  (9 = 8 dims + ones row)           (tiny)
  - keys_aug^T [32, 9, 4096] (row 8 = -|k|^2/2)              (4.6 MiB stream)
  - values   [32*4096, 8] in DRAM, randomly gathered at the end
Scores s[t,k] = x_t . k - |k|^2/2 computed on TensorE (argmax s == argmin dist);
argmax via VectorE max/max_index; final gather via indirect DMA.
"""

import numpy as np

import concourse.bass as bass
import concourse.tile as tile
from concourse import bacc, mybir
from concourse.bass_utils import run_bass_kernel_spmd

B, D, H, W = 8, 2048, 4, 4
C, K, d = 256, 4096, 8
NCORES = 8
CBC = C // NCORES          # 32 codebooks per core
T = B * H * W              # 128 tokens
DA = d + 1                 # augmented contraction dim (dot + bias row)
NK = K // 512              # 8 matmul chunks per codebook

F32 = mybir.dt.float32
U32 = mybir.dt.uint32


def build_program():
    nc = bacc.Bacc(trn_type="TRN2", num_devices=NCORES)

    xT = nc.dram_tensor("xT", [CBC, DA, T], F32, kind="ExternalInput")
    keysT = nc.dram_tensor("keysT", [CBC, DA, K], F32, kind="ExternalInput")
    vals = nc.dram_tensor("vals", [CBC * K, d], F32, kind="ExternalInput")
    base = nc.dram_tensor("base", [T, CBC], F32, kind="ExternalInput")
    out = nc.dram_tensor("out", [T, CBC * d], F32, kind="ExternalOutput")
    idx_out = nc.dram_tensor("idx_out", [T, CBC], U32, kind="ExternalOutput")

    with tile.TileContext(nc) as tc:
        with (
            tc.tile_pool(name="xsb", bufs=1) as x_pool,
            tc.tile_pool(name="kT", bufs=3) as kT_pool,
            tc.tile_pool(name="sc", bufs=2) as sc_pool,
            tc.tile_pool(name="small", bufs=4) as small_pool,
            tc.tile_pool(name="persist", bufs=1) as persist_pool,
            tc.tile_pool(name="psum", bufs=8, space="PSUM") as psum_pool,
        ):
            # all tokens' augmented x, transposed: [9, 32*128]
            x_sb = x_pool.tile([DA, CBC, T], F32)
            nc.sync.dma_start(x_sb[:], xT.ap().rearrange("c a t -> a c t"))

            base_sb = persist_pool.tile([T, CBC], F32)
            nc.sync.dma_start(base_sb[:], base.ap())

            idxf_all = persist_pool.tile([T, CBC], F32)

            for c in range(CBC):
                kT = kT_pool.tile([DA, K], F32)
                nc.sync.dma_start(kT[:], keysT.ap()[c])

                sc = sc_pool.tile([T, K], F32)
                for j in range(NK):
                    ps = psum_pool.tile([T, 512], F32)
                    nc.tensor.matmul(
                        ps[:],
                        lhsT=x_sb[:, c],
                        rhs=kT[:, j * 512:(j + 1) * 512],
                        start=True,
                        stop=True,
                    )
                    nc.any.tensor_copy(sc[:, j * 512:(j + 1) * 512], ps[:])

                mx8 = small_pool.tile([T, 8], F32)
                nc.vector.max(out=mx8[:], in_=sc[:])
                ix8 = small_pool.tile([T, 8], U32)
                nc.vector.max_index(ix8[:], mx8[:], sc[:])
                # stash argmax (slot 0) as f32 for the global-index add
                nc.vector.tensor_copy(idxf_all[:, c:c + 1], ix8[:, 0:1])

            # global row index into vals: idx + c*K
            idx_u = persist_pool.tile([T, CBC], U32)
            idxf2 = small_pool.tile([T, CBC], F32)
            nc.vector.tensor_add(idxf2[:], idxf_all[:], base_sb[:])
            nc.vector.tensor_copy(idx_u[:], idxf2[:])

            g = persist_pool.tile([T, CBC, d], F32)
            for c in range(CBC):
                nc.gpsimd.indirect_dma_start(
                    out=g[:, c],
                    out_offset=None,
                    in_=vals.ap(),
                    in_offset=bass.IndirectOffsetOnAxis(ap=idx_u[:, c:c + 1], axis=0),
                    bounds_check=CBC * K - 1,
                    oob_is_err=False,
                )
            nc.sync.dma_start(out.ap(), g[:].rearrange("t c dd -> t (c dd)"))
            nc.sync.dma_start(idx_out.ap(), idx_u[:])

    nc.compile()
    return nc


def make_core_inputs(embeddings: np.ndarray, keys: np.ndarray, values: np.ndarray):
    """Host-side shard prep. Returns list of input dicts, one per core."""
    # tokens: [B, D, H, W] -> [B*N, C, d]
    x = embeddings.reshape(B, D, H * W).transpose(0, 2, 1).reshape(T, C, d)
    # x_aug^T per codebook: [C, DA, T]
    xT = np.empty((C, DA, T), dtype=np.float32)
    xT[:, :d, :] = x.transpose(1, 2, 0)
    xT[:, d, :] = 1.0
    # keys_aug^T per codebook: [C, DA, K], row d = -|k|^2/2
    keysT = np.empty((C, DA, K), dtype=np.float32)
    keysT[:, :d, :] = keys.transpose(0, 2, 1)
    keysT[:, d, :] = -0.5 * np.einsum("ckd,ckd->ck", keys, keys)

    basec = (np.arange(CBC, dtype=np.float32) * K)[None, :].repeat(T, axis=0)
    basec = np.ascontiguousarray(basec)

    in_maps = []
    for i in range(NCORES):
        s = slice(i * CBC, (i + 1) * CBC)
        in_maps.append({
            "xT": np.ascontiguousarray(xT[s]),
            "keysT": np.ascontiguousarray(keysT[s]),
            "vals": np.ascontiguousarray(values[s].reshape(CBC * K, d)),
            "base": basec,
        })
    return in_maps


def assemble_output(results: list) -> np.ndarray:
    """results[i]["out"] is [T, CBC*d] for core i; -> [B, D, H, W]."""
    mem = np.concatenate(
        [r["out"].reshape(T, CBC * d) for r in results], axis=1
    )  # [T, C*d] == [B*N, D]
    return (
        mem.reshape(B, H * W, D).transpose(0, 2, 1).reshape(B, D, H, W)
    ).astype(np.float32)


_CACHED_NC = None


def kernel(embeddings, keys, values):
    global _CACHED_NC
    embeddings = np.asarray(embeddings, dtype=np.float32)
    keys = np.asarray(keys, dtype=np.float32)
    values = np.asarray(values, dtype=np.float32)
    if _CACHED_NC is None:
        _CACHED_NC = build_program()
    in_maps = make_core_inputs(embeddings, keys, values)
    res = run_bass_kernel_spmd(_CACHED_NC, in_maps, list(range(NCORES)))
    return assemble_output(res.results)


if __name__ == "__main__":
    rng = np.random.default_rng(0)
    emb = rng.standard_normal((B, D, H, W), dtype=np.float32)
    ks = rng.standard_normal((C, K, d), dtype=np.float32)
    vs = rng.standard_normal((C, K, d), dtype=np.float32)
    out = kernel(emb, ks, vs)
    print("out", out.shape, out.dtype, out.ravel()[:4])


# revision 10
# speedup vs baseline: 1.5955x; 1.5955x over previous
"""VQ codebook (DKVB) kernel for Trainium2, sharded over 8 NeuronCores.

Problem: embeddings [8, 2048, 4, 4] -> tokens x [128, 256, 8]; per codebook c
(256 of them), find nearest code among 4096 (euclidean), gather values row.

Strategy: shard the 256 codebooks across 8 cores (32 per core).
Score s[t,k] = x_t . k - |k|^2/2 (argmax s == argmin dist). Computed as an
exact-enough bf16 Dekker split (x = xh + xl, k = kh + kl, s = sh + sl):
    s = xh.kh + xh.kl + xl.kh - sh - sl   (error ~ 2^-17, fp32-grade)
folded into ONE bf16 matmul with contraction dim 26:
    lhsT rows = [xh(8); 1; xh(8); 1; xl(8)]
    rhs  rows = [kh(8); -sh; kl(8); -sl; kh(8)]
Argmax over 4096 codes in ONE VectorE pass with a custom DVE op:
    body = select(Src0 >= scan(MAX, Src0), Idx, -FLT_MAX), accum = MAX
(the last position that establishes a new running max == the argmax),
read directly from PSUM. Values gathered via per-codebook indirect DMA.
"""

import numpy as np
import ml_dtypes

import concourse.bass as bass
import concourse.tile as tile
from concourse import bacc, mybir
from concourse.bass_utils import run_bass_kernel_spmd

B, D, H, W = 8, 2048, 4, 4
C, K, d = 256, 4096, 8
NCORES = 8
CBC = C // NCORES          # 32 codebooks per core
T = B * H * W              # 128 tokens
KA = 26                    # Dekker-augmented contraction dim
NK = K // 512              # 8 matmul chunks per codebook

F32 = mybir.dt.float32
BF16 = mybir.dt.bfloat16
U32 = mybir.dt.uint32

_FLT_MAX = np.float32(3.4028235e38)


def _register_argmax_op():
    """Register the single-pass argmax custom DVE op (idempotent)."""
    from concourse import dve_ops
    from concourse.dve_spec import (
        Spec, Src0, MaxNeg, Idx, AluOp, select, lower, maxx, scan,
    )
    from concourse.dve_uop import DveOpSpec

    for op in dve_ops.OPS:
        if op.name == "ARGMAX_SCAN_ANT":
            return op

    def _ref(in0, in1, s0, s1, imm2):
        s = in0.astype(np.float32)
        r = np.maximum.accumulate(s, axis=-1)
        idx = np.broadcast_to(
            np.arange(s.shape[-1], dtype=np.float32), s.shape
        )
        body = np.where(s >= r, idx, -_FLT_MAX).astype(np.float32)
        acc = body.reshape(body.shape[0], -1).max(axis=-1, keepdims=True)
        return body, acc

    spec = Spec(
        body=select(Src0 >= scan(AluOp.MAX, Src0), Idx, MaxNeg),
        accum=maxx,
        reference=_ref,
    )
    opcode = dve_ops._CUSTOM_DVE_ROW_BASE + len(dve_ops.OPS)
    shas = {}
    for ver in ("v3", "v4"):
        s = DveOpSpec(name="ARGMAX_SCAN_ANT", opcode=opcode,
                      uops=lower(spec, ver=ver), rd1_en=False)
        shas[ver] = s.sha(ver)
    op = dve_ops.DveOp("ARGMAX_SCAN_ANT", spec, subdim=False, uops_sha=shas)
    dve_ops.OPS.append(op)
    dve_ops._SUB_OPCODE_FOR_NAME[op.name] = opcode
    dve_ops.CUSTOM_DVE_SPECS[op.name] = spec
    return op


def build_program():
    amax_op = _register_argmax_op()
    nc = bacc.Bacc(trn_type="TRN2", num_devices=NCORES)

    xT = nc.dram_tensor("xT", [CBC, KA, T], BF16, kind="ExternalInput")
    keysT = nc.dram_tensor("keysT", [CBC, KA, K], BF16, kind="ExternalInput")
    vals = nc.dram_tensor("vals", [CBC * K, d], F32, kind="ExternalInput")
    base = nc.dram_tensor("base", [T, CBC], F32, kind="ExternalInput")
    out = nc.dram_tensor("out", [T, CBC * d], F32, kind="ExternalOutput")
    idx_out = nc.dram_tensor("idx_out", [T, CBC], U32, kind="ExternalOutput")

    with tile.TileContext(nc) as tc:
        with (
            tc.tile_pool(name="xsb", bufs=1) as x_pool,
            tc.tile_pool(name="kT", bufs=3) as kT_pool,
            tc.tile_pool(name="scratch", bufs=1) as scr_pool,
            tc.tile_pool(name="small", bufs=4) as small_pool,
            tc.tile_pool(name="persist", bufs=1) as persist_pool,
            tc.tile_pool(name="psum", bufs=1, space="PSUM") as psum_pool,
        ):
            # all codebooks' Dekker-augmented xT: [26, CBC, T] bf16
            x_sb = x_pool.tile([KA, CBC, T], BF16)
            nc.sync.dma_start(x_sb[:], xT.ap().rearrange("c a t -> a c t"))

            base_sb = persist_pool.tile([T, CBC], F32)
            nc.sync.dma_start(base_sb[:], base.ap())

            idxf_all = persist_pool.tile([T, CBC], F32)
            scratch = scr_pool.tile([T, K], F32)

            for c in range(CBC):
                kT = kT_pool.tile([KA, K], BF16)
                nc.sync.dma_start(kT[:], keysT.ap()[c])

                ps = psum_pool.tile([T, K], F32)
                for j in range(NK):
                    nc.tensor.matmul(
                        ps[:, j * 512:(j + 1) * 512],
                        lhsT=x_sb[:, c],
                        rhs=kT[:, j * 512:(j + 1) * 512],
                        start=True,
                        stop=True,
                    )
                # single-pass argmax over all 4096 scores, straight from PSUM
                nc.vector._custom_dve(
                    amax_op,
                    out=scratch[:],
                    in0=ps[:],
                    accum_out=idxf_all[:, c:c + 1],
                )

            # global row index into vals: idx + c*K
            idx_u = persist_pool.tile([T, CBC], U32)
            idxf2 = small_pool.tile([T, CBC], F32)
            nc.vector.tensor_add(idxf2[:], idxf_all[:], base_sb[:])
            nc.vector.tensor_copy(idx_u[:], idxf2[:])
            nc.sync.dma_start(idx_out.ap(), idx_u[:])

            g = persist_pool.tile([T, CBC, d], F32)
            for c in range(CBC):
                nc.gpsimd.indirect_dma_start(
                    out=g[:, c],
                    out_offset=None,
                    in_=vals.ap(),
                    in_offset=bass.IndirectOffsetOnAxis(ap=idx_u[:, c:c + 1], axis=0),
                    bounds_check=CBC * K - 1,
                    oob_is_err=False,
                )
            nc.sync.dma_start(out.ap(), g[:].rearrange("t c dd -> t (c dd)"))

    nc.compile()
    return nc


def _bf16_split(a: np.ndarray):
    """Dekker split: a ~= hi + lo with hi, lo exactly representable in bf16."""
    hi = a.astype(ml_dtypes.bfloat16)
    lo = (a - hi.astype(np.float32)).astype(ml_dtypes.bfloat16)
    return hi, lo


def make_core_inputs(embeddings: np.ndarray, keys: np.ndarray, values: np.ndarray):
    """Host-side shard prep. Returns list of input dicts, one per core."""
    # tokens: [B, D, H, W] -> [B*N, C, d]
    x = embeddings.reshape(B, D, H * W).transpose(0, 2, 1).reshape(T, C, d)
    xh, xl = _bf16_split(np.ascontiguousarray(x))
    # lhsT rows: [xh(8); 1; xh(8); 1; xl(8)] -> [C, 26, T]
    xT = np.empty((C, KA, T), dtype=ml_dtypes.bfloat16)
    xT[:, 0:8] = xh.transpose(1, 2, 0)
    xT[:, 8] = 1.0
    xT[:, 9:17] = xT[:, 0:8]
    xT[:, 17] = 1.0
    xT[:, 18:26] = xl.transpose(1, 2, 0)

    kh, kl = _bf16_split(keys)
    s = 0.5 * np.einsum("ckd,ckd->ck", keys, keys)
    sh, sl = _bf16_split(s)
    # rhs rows: [kh(8); -sh; kl(8); -sl; kh(8)] -> [C, 26, K]
    keysT = np.empty((C, KA, K), dtype=ml_dtypes.bfloat16)
    keysT[:, 0:8] = kh.transpose(0, 2, 1)
    keysT[:, 8] = -sh
    keysT[:, 9:17] = kl.transpose(0, 2, 1)
    keysT[:, 17] = -sl
    keysT[:, 18:26] = keysT[:, 0:8]

    basec = (np.arange(CBC, dtype=np.float32) * K)[None, :].repeat(T, axis=0)
    basec = np.ascontiguousarray(basec)

    in_maps = []
    for i in range(NCORES):
        sl_ = slice(i * CBC, (i + 1) * CBC)
        in_maps.append({
            "xT": np.ascontiguousarray(xT[sl_]),
            "keysT": np.ascontiguousarray(keysT[sl_]),
            "vals": np.ascontiguousarray(values[sl_].reshape(CBC * K, d)),
            "base": basec,
        })
    return in_maps


def assemble_output(results: list) -> np.ndarray:
    """results[i]["out"] is [T, CBC*d] for core i; -> [B, D, H, W]."""
    mem = np.concatenate(
        [np.asarray(r["out"]).reshape(T, CBC * d) for r in results], axis=1
    )  # [T, C*d] == [B*N, D]
    return (
        mem.reshape(B, H * W, D).transpose(0, 2, 1).reshape(B, D, H, W)
    ).astype(np.float32)


_CACHED_NC = None


def kernel(embeddings, keys, values):
    global _CACHED_NC
    embeddings = np.asarray(embeddings, dtype=np.float32)
    keys = np.asarray(keys, dtype=np.float32)
    values = np.asarray(values, dtype=np.float32)
    if _CACHED_NC is None:
        _CACHED_NC = build_program()
    in_maps = make_core_inputs(embeddings, keys, values)
    res = run_bass_kernel_spmd(_CACHED_NC, in_maps, list(range(NCORES)))
    return assemble_output(res.results)


if __name__ == "__main__":
    rng = np.random.default_rng(0)
    emb = rng.standard_normal((B, D, H, W), dtype=np.float32)
    ks = rng.standard_normal((C, K, d), dtype=np.float32)
    vs = rng.standard_normal((C, K, d), dtype=np.float32)
    out = kernel(emb, ks, vs)
    print("out", out.shape, out.dtype, out.ravel()[:4])
